# revision 26
# baseline (speedup 1.0000x reference)
"""GQA attention (32 q heads / 8 kv heads, D=64, HID=2048, B=2, T=2048)
distributed over 8 TRN2 NeuronCores.

Sharding: 2-way data parallel (batch) x 4-way tensor parallel (head groups).
Core c handles batch c//4 and head group g=c%4 (q heads [8g,8g+8), kv heads
[2g,2g+2)).  Each core projects Q^T/K^T (transposed layout: head-dims on
partitions, T on free axis) and V (T on partitions), applies RoPE, computes
scores^T = K @ Q^T per head with keys on partitions, exp via ScalarE
(no max-subtraction needed at these magnitudes; masked entries multiply to
exactly 0 by a host-precomputed exp(mask) factor; fully-masked leading query
columns of diagonal tiles are skipped in scores/exp and zero-filled), then
out^T = Vext^T @ P^T where Vext carries a ones column producing the softmax
denominators for free.  AllGather runs per (q-tile, head-pair) — 16 small
gathers pipelined behind attention so the last one only covers the final
head-pair — feeding a local o_proj slice (512 output columns per core,
o_w rows permuted host-side to match the per-head-pair gather layout).

Everything matmul-facing is bf16 with fp32 PSUM accumulation.
"""

import os
import numpy as np
import ml_dtypes

BF16 = ml_dtypes.bfloat16

HQ, HKV, D, HID, THETA = 32, 8, 64, 2048, 10000.0
NCORES, NGROUPS = 8, 4
QDIM = HQ * D // NGROUPS        # 512 q dims per core
KVDIM = HKV * D // NGROUPS      # 128 kv dims per core
NQT = 512                       # query tile (free dim per PSUM bank)
NKC = 128                       # key chunk (partition dim)

_cache = {}
LAST_RESULT = None              # BassKernelResults of the most recent run


def plan_mask(mask, T):
    """Classify (key-chunk i, q-tile j) tiles of exp(mask).T.

    Returns (plans, emt_tiles): plans[j] = list of (i, kind, emt_idx, c0)
    where kind 0 = no mask needed (exp(mask)==1 on tile), kind 1 = multiply
    by emt_tiles[emt_idx]; c0 = first query column with any unmasked entry
    (columns [0, c0) are fully masked and skipped in scores/exp).  All-zero
    tiles are skipped entirely.
    """
    m = np.asarray(mask, dtype=np.float32).reshape(T, T)
    with np.errstate(under="ignore"):
        em = np.exp(m).T.astype(np.float32)   # em[k, q] = exp(mask[q, k])
    nj, nk = T // NQT, T // NKC
    plans, emt_tiles = [], []
    for j in range(nj):
        pj = []
        for i in range(nk):
            t = em[i * NKC:(i + 1) * NKC, j * NQT:(j + 1) * NQT]
            if not t.any():
                continue
            if (t == 1.0).all():
                pj.append((i, 0, -1, 0))
            else:
                c0 = int(np.argmax(t.any(axis=0)))
                pj.append((i, 1, len(emt_tiles), c0))
                emt_tiles.append(t.astype(BF16))
        plans.append(pj)
    return plans, emt_tiles


def build_graph(T, plans, n_emt):
    """Build the SPMD Bacc graph (same on all 8 cores; shards arrive as data)."""
    import concourse.bass as bass  # noqa: F401
    import concourse.mybir as mybir
    import concourse.tile as tile
    from concourse import bacc

    f32, bf16 = mybir.dt.float32, mybir.dt.bfloat16
    AF, ALU = mybir.ActivationFunctionType, mybir.AluOpType

    nj = T // NQT          # q tiles
    nhc = HID // 128       # contraction chunks over hidden dim
    noc = (HQ * D) // 128  # contraction chunks over gathered head dim (16)
    NE = max(n_emt, 1)

    nc = bacc.Bacc("TRN2", target_bir_lowering=False, debug=False,
                   num_devices=NCORES)

    xt = nc.dram_tensor("xt", [HID, T], bf16, kind="ExternalInput").ap()
    wqt = nc.dram_tensor("wqt", [HID, QDIM], bf16, kind="ExternalInput").ap()
    wkt = nc.dram_tensor("wkt", [HID, KVDIM], bf16, kind="ExternalInput").ap()
    wvt = nc.dram_tensor("wvt", [HID, KVDIM], bf16, kind="ExternalInput").ap()
    qb = nc.dram_tensor("qb", [4, 128, 1], f32, kind="ExternalInput").ap()
    kb = nc.dram_tensor("kb", [1, 128, 1], f32, kind="ExternalInput").ap()
    vb = nc.dram_tensor("vb", [1, KVDIM], bf16, kind="ExternalInput").ap()
    cosq = nc.dram_tensor("cosq", [128, T], bf16, kind="ExternalInput").ap()
    ssin = nc.dram_tensor("ssin", [128, T], bf16, kind="ExternalInput").ap()
    emt = nc.dram_tensor("emt", [NE, NKC, NQT], bf16,
                         kind="ExternalInput").ap()
    owt = nc.dram_tensor("owt", [HQ * D, QDIM], bf16, kind="ExternalInput").ap()
    out = nc.dram_tensor("out", [T, QDIM], f32, kind="ExternalOutput").ap()

    rg = [[0, 1, 2, 3], [4, 5, 6, 7]]

    with tile.TileContext(nc) as tc:
        with tc.tile_pool(name="dramp", bufs=1, space="DRAM") as dramp:
            # Per-(q-tile, head-pair) gather buffers: collectives freeze
            # the DMA subsystem while they run, so many short gathers beat
            # few long ones -- and the final gather covers only the last
            # head-pair.
            ag_in = [[dramp.tile([128, NQT], bf16, name=f"agin{j}_{hp}")
                      for hp in range(4)] for j in range(nj)]
            warm_in = dramp.tile([1, 64], bf16, name="warm_in")
            warm_out = dramp.tile([NGROUPS, 64], bf16, name="warm_out")
            ag_out = [[dramp.tile([NGROUPS * 128, NQT], bf16,
                                  name=f"agout{j}_{hp}")
                       for hp in range(4)] for j in range(nj)]

        with tc.tile_pool(name="persist", bufs=1) as pp:
            # Q^T per head-pair chunk: [128 (2 heads x 64), T]
            qt = [pp.tile([128, T], bf16, name=f"qt{m}") for m in range(4)]
            # K^T duplicated per kv head: [128 = kv dup'd twice, T]
            ktd = [pp.tile([128, T], bf16, name=f"ktd{k}") for k in range(2)]
            # V per key chunk: [128 keys, 130] (cols 0:64 kv0|64 ones|65:129 kv1|129 ones)
            vsb = [pp.tile([128, 130], bf16, name=f"v{i}")
                   for i in range(T // NKC)]
            ow_sb = pp.tile([128, noc, QDIM], bf16, name="ow_sb")
            emt_sb = pp.tile([128, NE, NQT], bf16, name="emt_sb")
            vb_sb = pp.tile([1, KVDIM], bf16, name="vb_sb")
            ones_sb = pp.tile([128, 128], bf16, name="ones_sb")
            qb_sb = pp.tile([128, 4], f32, name="qb_sb")
            kb_sb = pp.tile([128, 1], f32, name="kb_sb")

            nc.sync.dma_start(out=vb_sb[:], in_=vb)
            nc.vector.memset(ones_sb[:], 1.0)
            nc.sync.dma_start(out=qb_sb[:], in_=qb.rearrange("c p 1 -> p c"))
            nc.sync.dma_start(out=kb_sb[:], in_=kb.rearrange("c p 1 -> p c"))

            with tc.tile_pool(name="projw", bufs=1) as pw, \
                 tc.tile_pool(name="projx", bufs=2) as px, \
                 tc.tile_pool(name="projtmp", bufs=2) as ptmp, \
                 tc.tile_pool(name="pmain", bufs=2, space="PSUM") as pmain, \
                 tc.tile_pool(name="psS", bufs=2, space="PSUM") as psS, \
                 tc.tile_pool(name="psO", bufs=1, space="PSUM") as psO, \
                 tc.tile_pool(name="ptp", bufs=3) as ptp, \
                 tc.tile_pool(name="evp", bufs=2) as evp, \
                 tc.tile_pool(name="agp", bufs=8) as agp, \
                 tc.tile_pool(name="outp", bufs=4) as outp:
                wq_sb = pw.tile([128, nhc, QDIM], bf16, name="wq_sb")
                wk_sb = pw.tile([128, nhc, KVDIM], bf16, name="wk_sb")
                wv_sb = pw.tile([128, nhc, KVDIM], bf16, name="wv_sb")
                cos_sb = pw.tile([128, T], bf16, name="cos_sb")
                ssin_sb = pw.tile([128, T], bf16, name="ssin_sb")
                wqt_r = wqt.rearrange("(c p) q -> p c q", p=128)
                wkt_r = wkt.rearrange("(c p) q -> p c q", p=128)
                wvt_r = wvt.rearrange("(c p) q -> p c q", p=128)

                # grouped wq DMAs -> first matmuls start after the first
                # 512KB lands, without 16 serial trigger slots
                for g4 in range(4):
                    nc.sync.dma_start(out=wq_sb[:, 4 * g4:4 * (g4 + 1), :],
                                      in_=wqt_r[:, 4 * g4:4 * (g4 + 1), :])

                xt_r = xt.rearrange("(c p) t -> p c t", p=128)

                def rope_evict(ps, bias_col, dst, ts):
                    """dst = RoPE(ps + bias) cast to bf16."""
                    t2 = ptmp.tile([128, NQT], f32, name="t2", tag="t2")
                    nc.vector.scalar_tensor_tensor(
                        t2[:], ps[:], bias_col, ssin_sb[:, ts],
                        op0=ALU.add, op1=ALU.mult)
                    t2s = ptmp.tile([128, NQT], f32, name="t2s", tag="t2s")
                    for blk in range(4):
                        sb = blk ^ 1
                        nc.sync.dma_start(
                            out=t2s[blk * 32:(blk + 1) * 32, :],
                            in_=t2[sb * 32:(sb + 1) * 32, :])
                    t1 = ptmp.tile([128, NQT], f32, name="t1", tag="t1")
                    nc.vector.scalar_tensor_tensor(
                        t1[:], ps[:], bias_col, cos_sb[:, ts],
                        op0=ALU.add, op1=ALU.mult)
                    nc.vector.tensor_add(dst, t1[:], t2s[:])

                x_tiles = {}

                def proj_units(tb):
                    """Projection of T-block tb as a list of PE-dense units."""
                    ts = slice(tb * NQT, (tb + 1) * NQT)

                    def load_x():
                        quads = []
                        for q4 in range(4):
                            xq = px.tile([128, 4, NQT], bf16, name="x_sb",
                                         tag="x_sb", bufs=6)
                            nc.sync.dma_start(
                                out=xq[:],
                                in_=xt_r[:, q4 * 4:(q4 + 1) * 4, ts])
                            quads.append(xq)
                        x_tiles[tb] = quads

                    def qu(m):
                        def f():
                            xq = x_tiles[tb]
                            ps = pmain.tile([128, NQT], f32, name="ps",
                                            tag="ps")
                            for c in range(nhc):
                                nc.tensor.matmul(
                                    ps[:], wq_sb[:, c, m * 128:(m + 1) * 128],
                                    xq[c // 4][:, c % 4, :],
                                    start=(c == 0), stop=(c == nhc - 1))
                            rope_evict(ps, qb_sb[:, m:m + 1], qt[m][:, ts], ts)
                        return f

                    def ku():
                        xq = x_tiles[tb]
                        psk = pmain.tile([128, NQT], f32, name="psk", tag="ps")
                        for c in range(nhc):
                            nc.tensor.matmul(psk[:], wk_sb[:, c, :],
                                             xq[c // 4][:, c % 4, :],
                                             start=(c == 0),
                                             stop=(c == nhc - 1))
                        kf = ptmp.tile([128, NQT], bf16, name="kf", tag="kf")
                        rope_evict(psk, kb_sb[:, 0:1], kf[:], ts)
                        for half in (0, 1):
                            for dsth in (0, 1):
                                nc.sync.dma_start(
                                    out=ktd[half][dsth * 64:(dsth + 1) * 64,
                                                  ts],
                                    in_=kf[half * 64:(half + 1) * 64, :])

                    def vu(st):
                        def f():
                            xq = x_tiles[tb]
                            psv = pmain.tile([128, KVDIM], f32, name="psv",
                                             tag="ps")
                            for c in range(nhc):
                                nc.tensor.matmul(
                                    psv[:],
                                    xq[c // 4][:, c % 4,
                                               st * 128:(st + 1) * 128],
                                    wv_sb[:, c, :],
                                    start=(c == 0), stop=False)
                            nc.tensor.matmul(psv[:], ones_sb[0:1, :], vb_sb[:],
                                             start=False, stop=True)
                            vi = tb * (NQT // 128) + st
                            nc.vector.memset(vsb[vi][:, :], 1.0)
                            nc.vector.tensor_copy(vsb[vi][:, 0:64],
                                                  psv[:, 0:64])
                            nc.vector.tensor_copy(vsb[vi][:, 65:129],
                                                  psv[:, 64:128])
                        return f

                    return [load_x] + [qu(m) for m in range(4)] + [ku] + \
                        [vu(st) for st in range(NQT // 128)]

                ag_sb_tiles = {}

                def attn_core(j, hp):
                    qs0 = j * NQT
                    kv = hp // 2
                    po0 = psO.tile([65, NQT], f32, name="po0", tag="po0")
                    po1 = psO.tile([65, NQT], f32, name="po1", tag="po1")
                    n_ch = len(plans[j])
                    for ci, (i, kind, gi, c0) in enumerate(plans[j]):
                        pss = psS.tile([128, 1024], f32, name="pss", tag="pss")
                        qsc = slice(qs0 + c0, qs0 + NQT)
                        # head-lo on array rows 0:64, head-hi on 64:128 —
                        # concurrent row-groups, separate PSUM banks
                        nc.tensor.matmul(
                            pss[:, c0:512],
                            ktd[kv][0:64, i * NKC:(i + 1) * NKC],
                            qt[hp][0:64, qsc], start=True, stop=True)
                        nc.tensor.matmul(
                            pss[:, 512 + c0:1024],
                            ktd[kv][64:128, i * NKC:(i + 1) * NKC],
                            qt[hp][64:128, qsc], start=True, stop=True)
                        pt = ptp.tile([128, 1024], bf16, name="pt", tag="pt")
                        if c0 > 0:
                            nc.vector.memset(pt[:, 0:c0], 0.0)
                            nc.vector.memset(pt[:, 512:512 + c0], 0.0)
                            nc.scalar.activation(pt[:, c0:512],
                                                 pss[:, c0:512],
                                                 AF.Exp, scale=0.125)
                            nc.scalar.activation(pt[:, 512 + c0:1024],
                                                 pss[:, 512 + c0:1024],
                                                 AF.Exp, scale=0.125)
                        else:
                            nc.scalar.activation(pt[:], pss[:], AF.Exp,
                                                 scale=0.125)
                        if kind == 1:
                            nc.vector.tensor_mul(pt[:, c0:512], pt[:, c0:512],
                                                 emt_sb[:, gi, c0:512])
                            nc.vector.tensor_mul(pt[:, 512 + c0:1024],
                                                 pt[:, 512 + c0:1024],
                                                 emt_sb[:, gi, c0:512])
                        vsl = vsb[i][:, 0:65] if kv == 0 else vsb[i][:, 65:130]
                        nc.tensor.matmul(po0[:], vsl, pt[:, 0:512],
                                         start=(ci == 0), stop=(ci == n_ch - 1))
                        nc.tensor.matmul(po1[:], vsl, pt[:, 512:1024],
                                         start=(ci == 0), stop=(ci == n_ch - 1))
                    evs = []
                    for s, po in enumerate((po0, po1)):
                        # One fast copy frees the PSUM bank; the divide chain
                        # then runs off the PE critical path from SBUF.
                        pocp = evp.tile([65, NQT], f32, name="pocp",
                                        tag="pocp", bufs=4)
                        nc.vector.tensor_copy(pocp[:], po[:])
                        evs.append((s, pocp))

                    def evict():
                        # Deferred one head-pair so none of this ever parks at
                        # the head of a queue: spread the 512 denominators
                        # over 128 lanes (reciprocal is ~6 cyc/elem serial per
                        # partition), reciprocal, then partition-broadcast via
                        # a 1-row PE matmul.  No gpsimd involvement at all.
                        for s, pocp in evs:
                            rs = evp.tile([128, 4], f32, name="rs", tag="rs")
                            nc.sync.dma_start(out=rs[:], in_=pocp[64:65, :])
                            rr = evp.tile([128, 4], f32, name="rr", tag="rr")
                            nc.vector.reciprocal(rr[:], rs[:])
                            rrb = evp.tile([128, 4], bf16, name="rrb",
                                           tag="rrb")
                            nc.vector.tensor_copy(rrb[:], rr[:])
                            rc = evp.tile([1, NQT], bf16, name="rc", tag="rc")
                            nc.sync.dma_start(out=rc[:], in_=rrb[:])
                            rb = psS.tile([64, NQT], f32, name="rb",
                                          tag="pss")
                            nc.tensor.matmul(rb[:], ones_sb[0:1, 0:64],
                                             rc[:], start=True, stop=True)
                            at = evp.tile([64, NQT], bf16, name="at", tag="at",
                                          bufs=4)
                            nc.vector.tensor_mul(at[:], pocp[0:64, :], rb[:])
                            nc.sync.dma_start(
                                out=ag_in[j][hp][s * 64:(s + 1) * 64, :],
                                in_=at[:])
                        nc.gpsimd.collective_compute(
                            "AllGather", ALU.bypass, replica_groups=rg,
                            ins=[ag_in[j][hp].opt()],
                            outs=[ag_out[j][hp].opt()])
                    return evict

                def load_ag(j):
                    """Load the 4 gathered head-pair blocks of q-tile j into
                    SBUF.  Emitted only once the gathers are provably done so
                    the triggers never park at the head of a DMA queue."""
                    def f():
                        for hp in range(4):
                            ag_sb = agp.tile([128, 4, NQT], bf16,
                                             name="ag_sb", tag="ag_sb",
                                             bufs=8)
                            nc.sync.dma_start(
                                out=ag_sb[:],
                                in_=ag_out[j][hp].rearrange(
                                    "(c p) t -> p c t", p=128))
                            ag_sb_tiles[(j, hp)] = ag_sb
                    return f

                def oproj_units(j):
                    # chunk c (core-major: source core c//4, local head-pair
                    # c%4) lives in gather tile (j, hp=c%4) at core slot c//4.
                    # For the last q-tile, accumulate in hp-major order so
                    # only the final 4 matmuls wait on AllGather (last, 3).
                    if j == nj - 1:
                        order = [g * 4 + lhp for lhp in range(4)
                                 for g in range(NGROUPS)]
                    else:
                        order = list(range(noc))

                    def ou(tt):
                        def f():
                            pf = pmain.tile([128, QDIM], f32, name="pf",
                                            tag="ps")
                            for ci, c in enumerate(order):
                                nc.tensor.matmul(
                                    pf[:],
                                    ag_sb_tiles[(j, c % 4)][
                                        :, c // 4, tt * 128:(tt + 1) * 128],
                                    ow_sb[:, c, :],
                                    start=(ci == 0), stop=(ci == noc - 1))
                            ot = outp.tile([128, QDIM], f32, name="ot",
                                           tag="ot")
                            nc.vector.tensor_copy(ot[:], pf[:])
                            nc.sync.dma_start(
                                out=out[j * NQT + tt * 128:
                                        j * NQT + (tt + 1) * 128, :],
                                in_=ot[:])
                        return f
                    return [ou(tt) for tt in range(NQT // 128)]

                # ---------------- the schedule ----------------
                p0 = proj_units(0)
                p0[0]()                       # x(tb0) quads on sync queue
                # rope tables trigger from the (otherwise empty) gpsimd queue
                # so their transfers start at t=0; wk/wv/emt go on sync AFTER
                # the x quads so wq/x transfers get HBM bandwidth first
                nc.gpsimd.dma_start(out=cos_sb[:], in_=cosq)
                nc.gpsimd.dma_start(out=ssin_sb[:], in_=ssin)
                # tiny warmup gather: absorbs the first-collective rendezvous
                # and cross-core skew while the PE is busy with projections
                nc.gpsimd.collective_compute(
                    "AllGather", ALU.bypass, replica_groups=rg,
                    ins=[warm_in.opt()], outs=[warm_out.opt()])
                nc.sync.dma_start(out=wk_sb[:], in_=wkt_r)
                nc.sync.dma_start(out=wv_sb[:], in_=wvt_r)
                emt_r = emt.rearrange("n p q -> p n q")
                n0 = max((gi + 1 for (i, k, gi, c0) in plans[0] if k == 1),
                         default=0)
                if n0:
                    nc.sync.dma_start(out=emt_sb[:, 0:n0, :],
                                      in_=emt_r[:, 0:n0, :])
                for u in p0[1:]:
                    u()
                if n_emt > n0:
                    nc.sync.dma_start(out=emt_sb[:, n0:n_emt, :],
                                      in_=emt_r[:, n0:n_emt, :])
                pending = None
                for j in range(nj):
                    if j == 1:
                        # o_proj weights aren't needed until j=2; this trigger
                        # waits on nothing so it can't stall the scalar queue
                        nc.scalar.dma_start(
                            out=ow_sb[:],
                            in_=owt.rearrange("(c p) q -> p c q", p=128))
                    filler = []
                    if j + 1 < nj:
                        filler += proj_units(j + 1)
                    if j == 2:
                        filler += [load_ag(0)] + oproj_units(0)
                        filler += [load_ag(1)]
                    if j == 3:
                        filler += oproj_units(1) + [load_ag(2)]
                    nf = len(filler)
                    for hp in range(4):
                        # filler BEFORE each head-pair: keeps PE fed while the
                        # first scores of the block wait on rope/exp chains
                        for u in filler[nf * hp // 4:nf * (hp + 1) // 4]:
                            u()
                        ev = attn_core(j, hp)
                        if pending is not None:
                            pending()
                        pending = ev
                    if j < nj - 1:
                        # flush at the j boundary so AllGather (j,3) isn't
                        # held back into the next block
                        pending()
                        pending = None
                # tail: o_proj(2) first — it needs no gather anymore and its
                # matmuls keep PE busy while AllGather (3,2) freezes the DMA
                # queues that evict(3,3)'s reciprocal chain depends on.
                for u in oproj_units(nj - 2):
                    u()
                pending()
                pending = None
                load_ag(nj - 1)()
                # o_proj of the last q-tile with all 4 output-tile
                # accumulators live at once (attention PSUM banks are free
                # by now): the 48 hp0-2 chunk-matmuls are emitted first, so
                # only 16 matmuls sit behind the AllGather (3,3) gate
                # instead of 52.
                jl = nj - 1
                pfs = []
                for tt in range(4):
                    if tt < 2:
                        pf = pmain.tile([128, QDIM], f32, name="pfl",
                                        tag="ps")
                    else:
                        pf = psS.tile([128, QDIM], f32, name="pfl",
                                      tag="pss")
                    pfs.append(pf)
                for tt in range(4):
                    ci = 0
                    for lhp in range(3):
                        for g in range(NGROUPS):
                            c = g * 4 + lhp
                            nc.tensor.matmul(
                                pfs[tt][:],
                                ag_sb_tiles[(jl, c % 4)][
                                    :, c // 4, tt * 128:(tt + 1) * 128],
                                ow_sb[:, c, :],
                                start=(ci == 0), stop=False)
                            ci += 1
                for tt in range(4):
                    for k in range(NGROUPS):
                        c = k * 4 + 3
                        nc.tensor.matmul(
                            pfs[tt][:],
                            ag_sb_tiles[(jl, c % 4)][
                                :, c // 4, tt * 128:(tt + 1) * 128],
                            ow_sb[:, c, :],
                            start=False, stop=(k == NGROUPS - 1))
                    ot = outp.tile([128, QDIM], f32, name="ot", tag="ot")
                    nc.vector.tensor_copy(ot[:], pfs[tt][:])
                    nc.sync.dma_start(
                        out=out[jl * NQT + tt * 128:
                                jl * NQT + (tt + 1) * 128, :],
                        in_=ot[:])

    nc.compile()
    return nc


def prep_inputs(hidden, positions, mask, q_w, q_b, k_w, k_b, v_w, v_b, o_w,
                emt_tiles):
    """Host-side shard + transform -> in_maps for the 8 cores."""
    B, T, _ = hidden.shape
    pos = np.asarray(positions)[0].astype(np.float32)
    inv_freq = (1.0 / (THETA ** (np.arange(0, D, 2, dtype=np.float32) / D)))
    freqs = pos[:, None] * inv_freq[None, :]          # (T, 32)
    cos_t, sin_t = np.cos(freqs).T, np.sin(freqs).T   # (32, T)
    cos_tab = np.ascontiguousarray(np.tile(cos_t, (4, 1))).astype(BF16)
    ssin_tab = np.ascontiguousarray(
        np.concatenate([sin_t, -sin_t, sin_t, -sin_t], axis=0)).astype(BF16)

    if emt_tiles:
        emt_arr = np.stack(emt_tiles).astype(BF16)
    else:
        emt_arr = np.zeros((1, NKC, NQT), BF16)

    owt_full = np.ascontiguousarray(o_w.T)            # [HQ*D, HID]
    xts = [np.ascontiguousarray(hidden[b].T).astype(BF16) for b in range(B)]
    in_maps = []
    for c in range(NCORES):
        b, g = c // NGROUPS, c % NGROUPS
        qsl = slice(QDIM * g, QDIM * (g + 1))
        ksl = slice(KVDIM * g, KVDIM * (g + 1))
        owt_c = np.ascontiguousarray(owt_full[:, qsl]).astype(BF16)
        in_maps.append({
            "xt": xts[b],
            "wqt": np.ascontiguousarray(q_w[qsl, :].T).astype(BF16),
            "wkt": np.ascontiguousarray(k_w[ksl, :].T).astype(BF16),
            "wvt": np.ascontiguousarray(v_w[ksl, :].T).astype(BF16),
            "qb": np.asarray(q_b[qsl], np.float32).reshape(4, 128, 1),
            "kb": np.asarray(k_b[ksl], np.float32).reshape(1, 128, 1),
            "vb": np.asarray(v_b[ksl]).astype(BF16).reshape(1, KVDIM),
            "cosq": cos_tab,
            "ssin": ssin_tab,
            "emt": emt_arr,
            "owt": owt_c,
        })
    return in_maps


def _ensure_ntff_hook():
    """Provide antenv.axon_hooks in containers whose antenv stub lacks it,
    wiring the ctypes NTFF profiler from the injected axon boot package."""
    import sys
    import types
    try:
        from antenv.axon_hooks import get_axon_ntff_profile_hook  # noqa: F401
        return True
    except ImportError:
        pass
    try:
        import antenv
        from trn_agent_boot.trn_boot import _ntff_profile_via_ctypes
        hook = _ntff_profile_via_ctypes("/opt/axon/libaxon_pjrt.so")
        if hook is None:
            return False
        mod = types.ModuleType("antenv.axon_hooks")
        state = {"h": hook}
        mod.get_axon_ntff_profile_hook = lambda: state["h"]
        mod.set_axon_ntff_profile_hook = lambda h: state.__setitem__("h", h)
        sys.modules["antenv.axon_hooks"] = mod
        antenv.axon_hooks = mod
        return True
    except Exception:
        return False


def kernel(hidden, positions, mask, q_w, q_b, k_w, k_b, v_w, v_b, o_w):
    global LAST_RESULT
    from concourse import bass_utils

    hidden = np.asarray(hidden)
    B, T, _ = hidden.shape
    mask_key = (T, hash(np.asarray(mask).tobytes()))
    if mask_key not in _cache:
        plans, emt_tiles = plan_mask(mask, T)
        nc = build_graph(T, plans, len(emt_tiles))
        _cache[mask_key] = (nc, emt_tiles)
    nc, emt_tiles = _cache[mask_key]

    in_maps = prep_inputs(hidden, positions, mask, q_w, q_b, k_w, k_b,
                          v_w, v_b, o_w, emt_tiles)
    trace = os.environ.get("BASS_KERNEL_TRACE", "0") == "1"
    if trace:
        trace = _ensure_ntff_hook()
    res = bass_utils.run_bass_kernel_spmd(nc, in_maps,
                                          core_ids=list(range(NCORES)),
                                          trace=trace)
    LAST_RESULT = res
    out = np.zeros((B, T, HID), np.float32)
    for c in range(NCORES):
        b, g = c // NGROUPS, c % NGROUPS
        out[b, :, QDIM * g:QDIM * (g + 1)] = res.results[c]["out"]
    return out
